# revision 15
# baseline (speedup 1.0000x reference)
"""GAT kernel for TRN2: host prep + Bass program builder.

Sharding: nodes (and their in-edges) partitioned across 8 cores by contiguous
shard; the per-layer feature/attention table (rows [h bf16(256B) | al f32(16B)
| pad], 512B stride) is exchanged via C chunked AllGathers that pipeline with
the dense phase; per dst-block-of-128 selector-matmul scatter with edge
gathers (gpsimd dma_gather, int16 indices into per-chunk tables); BN via
AllReduce of partial sums; pooling via graph-selector matmul; tiny FC + final
AllReduce.
"""
from dataclasses import dataclass, field

import numpy as np

import concourse.bacc as bacc
import concourse.bass as bass
import concourse.mybir as mybir
import concourse.tile as tile
from concourse import library_config

F32 = mybir.dt.float32
BF = mybir.dt.bfloat16
I16 = mybir.dt.int16
I8 = mybir.dt.int8
AX = mybir.AluOpType
AF = mybir.ActivationFunctionType


class _SkipRest(Exception):
    pass


@dataclass
class Cfg:
    ncores: int = 8
    n_real: int = 50000       # real nodes
    np_: int = 50176          # padded nodes (multiple of ncores*128)
    e_raw: int = 800000       # edges before self loops
    g: int = 500              # graphs
    gp: int = 512             # padded graphs (pool matmul free dim)
    f: int = 128              # features (in = out = 128)
    h: int = 4
    c: int = 32
    k: int = 6
    eps: float = 1e-5
    rowf: int = 128           # table row stride in f32 (512B; gather elem)
    stage: int = 9            # debug: how much of the program to emit
    repeat: int = 1           # timing: execute the whole body N times
    no_ag: bool = False       # debug/bench: skip table AllGathers
    ag_chunks: int = 2        # table AllGather split into C row-chunks
    sel_dram: bool = False    # load host-precomputed one-hot sel from DRAM
    neg_pad: bool = False     # pad gather idx with -1 (skips the DMA)
    pool_selt: int = 0        # build selT on Pool engine every Nth block
    neg_slope: float = 0.2
    # edge packing metadata (filled by prep_edges)
    tbc: tuple = ()           # [nblk][C] tiles per (dst block, src chunk)
    tb: tuple = ()            # [nblk] total tiles per block
    tbmax: int = 0
    ioff: tuple = ()          # [nblk+1] tile prefix offsets
    cbc: tuple = ()           # [nblk][C] max real idx count per bucket
    bf16_rows: bool = True    # kept for test.py compat (always on)
    bf16_mm: bool = True

    @property
    def shard(self):
        return self.np_ // self.ncores

    @property
    def nblk(self):
        return self.shard // 128

    @property
    def rin(self):
        return self.shard // self.ag_chunks   # rows per (core, chunk)

    @property
    def crows(self):
        return self.np_ // self.ag_chunks     # rows per chunk tensor


def fold_attn(a, H, C):
    A = np.zeros((H * C, H), np.float32)
    for h in range(H):
        A[h * C:(h + 1) * C, h] = a[h]
    return A


def pack_idx16(idx, slots, pad=0):
    """idx list -> [128, slots//16] int16 dma_gather layout.

    pad=-1 makes the DMA skip the padding slots entirely.
    """
    arr = np.full((16, slots // 16), pad, dtype=np.int16)
    j = np.arange(len(idx))
    arr[j % 16, j // 16] = idx
    return np.tile(arr, (8, 1))


def prep_edges(cfg: Cfg, edge_index):
    """Bucket edges by (dst core, dst block, src chunk); fill cfg tile counts.

    Returns per-core `packed[ci] = (srcp, dstp)` where srcp/dstp are
    [tot_tiles, 128] int64 ragged-by-block arrays (src row within its chunk
    tensor; dst lane 0..127 or 999 sentinel).
    """
    n, sh = cfg.n_real, cfg.shard
    C = cfg.ag_chunks
    rin = cfg.rin
    src = np.concatenate([edge_index[0], np.arange(n)]).astype(np.int64)
    dst = np.concatenate([edge_index[1], np.arange(n)]).astype(np.int64)
    s_core = src // sh
    s_loc = src % sh
    s_chunk = s_loc // rin
    s_row = s_core * rin + (s_loc % rin)

    NB = cfg.nblk
    buckets = [[[None] * C for _ in range(NB)] for _ in range(cfg.ncores)]
    tbc = [[1 if C == 1 else 0] * C for _ in range(NB)]
    cbc = [[0] * C for _ in range(NB)]
    for ci in range(cfg.ncores):
        m = (dst // sh) == ci
        d = dst[m] - ci * sh
        sc, sr = s_chunk[m], s_row[m]
        for b in range(NB):
            mb = (d // 128) == b
            db = d[mb] % 128
            scb, srb = sc[mb], sr[mb]
            for c in range(C):
                mc = scb == c
                buckets[ci][b][c] = (srb[mc], db[mc])
                tbc[b][c] = max(tbc[b][c], -(-int(mc.sum()) // 128))
                cbc[b][c] = max(cbc[b][c], int(mc.sum()))
    cfg.cbc = tuple(tuple(r) for r in cbc)
    cfg.tbc = tuple(tuple(r) for r in tbc)
    cfg.tb = tuple(sum(r) for r in tbc)
    cfg.tbmax = max(cfg.tb)
    ioff = [0]
    for b in range(NB):
        ioff.append(ioff[-1] + cfg.tb[b])
    cfg.ioff = tuple(ioff)

    packed = []
    src_pad = -1 if cfg.neg_pad else 0
    for ci in range(cfg.ncores):
        srcp = np.full((cfg.ioff[-1], 128), src_pad, np.int64)
        dstp = np.full((cfg.ioff[-1], 128), 999, np.int64)
        for b in range(NB):
            off = cfg.ioff[b]
            for c in range(C):
                t = cfg.tbc[b][c]
                s, d = buckets[ci][b][c]
                srcp[off:off + t].flat[: len(s)] = s
                # equalize non-negative idx count across cores (num_idxs_reg)
                srcp[off:off + t].flat[len(s): cfg.cbc[b][c]] = 0
                dstp[off:off + t].flat[: len(d)] = d
                off += t
        packed.append((srcp, dstp))
    return packed


def prep_inputs(cfg: Cfg, inputs):
    """Build per-core in_maps (list of dicts)."""
    H, C_, F = cfg.h, cfg.c, cfg.f
    packed = prep_edges(cfg, inputs["edge_index"])
    NB, TOT, TBM = cfg.nblk, cfg.ioff[-1], cfg.tbmax

    xpad = np.zeros((cfg.np_, F), np.float32)
    xpad[: cfg.n_real] = inputs["x"]

    W1e = np.concatenate(
        [inputs["W1"], inputs["W1"] @ fold_attn(inputs["a_src1"], H, C_),
         inputs["W1"] @ fold_attn(inputs["a_dst1"], H, C_)], axis=1)  # [F,136]
    W2e = np.concatenate(
        [inputs["W2"], inputs["W2"] @ fold_attn(inputs["a_src2"], H, C_),
         inputs["W2"] @ fold_attn(inputs["a_dst2"], H, C_)], axis=1)

    batch = np.asarray(inputs["batch"]).astype(np.int64)
    batch_pad = np.full(cfg.np_, 999, np.int64)
    batch_pad[: cfg.n_real] = batch
    cnt = np.bincount(batch, minlength=cfg.gp).astype(np.float32)
    rcnt = (1.0 / np.maximum(cnt, 1.0)).astype(np.float32)
    bf = mybir.dt.np(BF)

    shared = dict(
        w1e=W1e.astype(np.float32), w2e=W2e.astype(np.float32),
        b1bc=np.tile(inputs["b1"][None, :], (128, 1)).astype(np.float32),
        b2bc=np.tile(inputs["b2"][None, :], (128, 1)).astype(np.float32),
        g1row=inputs["g1"][None, :].astype(np.float32),
        be1row=inputs["be1"][None, :].astype(np.float32),
        g2row=inputs["g2"][None, :].astype(np.float32),
        be2row=inputs["be2"][None, :].astype(np.float32),
        fcw=inputs["fcW"].astype(np.float32),
        fcbbc=np.tile(inputs["fcb"][:, None], (1, cfg.gp)).astype(np.float32),
        rcntbc=np.tile(rcnt[None, :], (cfg.k, 1)).astype(np.float32),
        ident=np.eye(128, dtype=np.float32),
        diota=np.tile(np.arange(128, dtype=np.float32)[None, :], (128, 1)).astype(bf),
        piota=np.arange(128, dtype=np.float32)[:, None].copy(),
        giota=np.tile(np.arange(cfg.gp, dtype=np.float32)[None, :], (128, 1)),
        onescol=np.ones((128, 1), np.float32),
        onesrow=np.ones((1, 128), np.float32),
    )

    in_maps = []
    for ci in range(cfg.ncores):
        srcp, dstp = packed[ci]
        idx_flat = np.zeros((128, TOT * 8), np.int16)
        for b in range(NB):
            off = cfg.ioff[b]
            for c in range(cfg.ag_chunks):
                t = cfg.tbc[b][c]
                if t:
                    idx_flat[:, off * 8:(off + t) * 8] = pack_idx16(
                        srcp[off:off + t].reshape(-1), t * 128)
                off += t
        # dstsel [128 lane, NB, TBM] bf16 (999 sentinel)
        dstsel = np.full((128, NB, TBM), 999, np.float32)
        # drow [NB, 128, TBM*128] int8 (-1 sentinel)
        drow = np.full((NB, TBM * 128), -1, np.int8)
        selp = (np.zeros((NB, 128, TBM * 128), bf) if cfg.sel_dram
                else np.zeros((1,), bf))
        lanes = np.arange(128)
        for b in range(NB):
            tb = cfg.tb[b]
            blk = dstp[cfg.ioff[b]: cfg.ioff[b] + tb]       # [tb, 128]
            dstsel[:, b, :tb] = blk.T
            drow[b, : tb * 128] = np.where(blk > 127, -1, blk).reshape(-1)
            if cfg.sel_dram:
                # sel[e, t*128+c] = (dst(b,t,e) == c)
                eq = (blk[:, :, None] == lanes[None, None, :])   # [tb, e, c]
                selp[b, :, : tb * 128] = np.ascontiguousarray(
                    eq.transpose(1, 0, 2)).reshape(128, tb * 128).astype(bf)
        drow = np.broadcast_to(drow[:, None, :], (NB, 128, TBM * 128)).copy()

        sl = slice(ci * cfg.shard, (ci + 1) * cfg.shard)
        nm = np.zeros((128, NB), np.float32)
        bc = np.zeros((128, NB), np.float32)
        ids = np.arange(ci * cfg.shard, (ci + 1) * cfg.shard)
        nm[:] = (ids.reshape(NB, 128).T < cfg.n_real)
        bc[:] = batch_pad[ids].reshape(NB, 128).T.astype(np.float32)
        xs = xpad[sl].reshape(NB, 128, F).transpose(1, 0, 2)
        in_maps.append(dict(
            x_shard=np.ascontiguousarray(xs).reshape(128, NB * F),
            idx_flat=idx_flat,
            dstsel=dstsel.astype(bf),
            dstrow=drow,
            selp=selp,
            node_mask=nm, batchcol=bc,
            **shared,
        ))
    return in_maps


# ---------------------------------------------------------------------------
# Bass program
# ---------------------------------------------------------------------------

def build_nc(cfg: Cfg):
    NB, TBM, TOT = cfg.nblk, cfg.tbmax, cfg.ioff[-1]
    F, H, C_, RF = cfg.f, cfg.h, cfg.c, cfg.rowf
    FH = F + H
    SH = cfg.shard
    GP = cfg.gp
    CC = cfg.ag_chunks
    ALO = 64                  # f32-slot offset of al in a table row

    nc = bacc.Bacc("TRN2", target_bir_lowering=False, debug=False,
                   num_devices=cfg.ncores, num_swdge_queues=4)

    def ext(name, shape, dtype=F32):
        return nc.dram_tensor(name, shape, dtype, kind="ExternalInput")

    x_shard = ext("x_shard", [128, NB * F])
    idx_flat_d = ext("idx_flat", [128, TOT * 8], I16)
    if cfg.sel_dram:
        selp_d = ext("selp", [NB, 128, TBM * 128], BF)
    else:
        dstsel_d = ext("dstsel", [128, NB, TBM], BF)
    dstrow_d = ext("dstrow", [NB, 128, TBM * 128], I8)
    node_mask = ext("node_mask", [128, NB])
    batchcol = ext("batchcol", [128, NB])
    w1e = ext("w1e", [F, F + 2 * H])
    w2e = ext("w2e", [F, F + 2 * H])
    b1bc = ext("b1bc", [128, F])
    b2bc = ext("b2bc", [128, F])
    g1row = ext("g1row", [1, F])
    be1row = ext("be1row", [1, F])
    g2row = ext("g2row", [1, F])
    be2row = ext("be2row", [1, F])
    fcw = ext("fcw", [F, cfg.k])
    fcbbc = ext("fcbbc", [cfg.k, GP])
    rcntbc = ext("rcntbc", [cfg.k, GP])
    ident_d = ext("ident", [128, 128])
    diota_d = ext("diota", [128, 128], BF)
    piota_d = ext("piota", [128, 1])
    giota_d = ext("giota", [128, GP])
    onescol_d = ext("onescol", [128, 1])
    onesrow_d = ext("onesrow", [1, 128])

    out_d = nc.dram_tensor("out", [cfg.k, GP], F32, kind="ExternalOutput")

    rg = [list(range(cfg.ncores))]
    shared_as = "Shared" if cfg.ncores > 4 else "Local"

    with tile.TileContext(nc) as tc:
        with (
            tc.tile_pool(name="dram", bufs=1, space="DRAM") as dpool,
            tc.tile_pool(name="persist", bufs=1) as pp,
            tc.tile_pool(name="consts", bufs=1) as cp,
            tc.tile_pool(name="work", bufs=3) as wp_pool,
            tc.tile_pool(name="gath", bufs=3) as gp_pool,
            tc.tile_pool(name="psum", bufs=3, space="PSUM") as ps_pool,
            tc.tile_pool(name="psum1", bufs=1, space="PSUM") as ps1_pool,
        ):
            nc.gpsimd.load_library(library_config.mlp)

            # ---- persistent SBUF ----
            h_cur = pp.tile([128, NB, F], F32)          # shard activations
            ar_sb = pp.tile([128, NB, H], BF)
            dstsel_sb = (None if cfg.sel_dram
                         else pp.tile([128, NB, TBM], BF))
            mask_sb = pp.tile([128, NB], F32)
            bcol_sb = pp.tile([128, NB], F32)

            # ---- constants ----
            w1e_sb = cp.tile([128, F + 2 * H], F32)
            w2e_sb = cp.tile([128, F + 2 * H], F32)
            b1bc_sb = cp.tile([128, F], F32)
            b2bc_sb = cp.tile([128, F], F32)
            ident = cp.tile([128, 128], F32)
            diota = cp.tile([128, 128], BF)
            piota = cp.tile([128, 1], F32)
            giota = cp.tile([128, GP], F32)
            onescol = cp.tile([128, 1], F32)
            onesrow = cp.tile([1, 128], F32)
            g1_sb = cp.tile([1, F], F32)
            be1_sb = cp.tile([1, F], F32)
            g2_sb = cp.tile([1, F], F32)
            be2_sb = cp.tile([1, F], F32)
            fcw_sb = cp.tile([128, cfg.k], F32)
            fcbbc_sb = cp.tile([cfg.k, GP], F32)
            rcnt_sb = cp.tile([cfg.k, GP], F32)

            for sb, d in [(w1e_sb, w1e), (w2e_sb, w2e), (b1bc_sb, b1bc),
                          (b2bc_sb, b2bc), (ident, ident_d), (diota, diota_d),
                          (piota, piota_d), (giota, giota_d),
                          (onescol, onescol_d), (onesrow, onesrow_d),
                          (g1_sb, g1row), (be1_sb, be1row),
                          (g2_sb, g2row), (be2_sb, be2row), (fcw_sb, fcw),
                          (fcbbc_sb, fcbbc), (rcnt_sb, rcntbc),
                          (mask_sb, node_mask), (bcol_sb, batchcol),
                          (h_cur, x_shard)] + (
                          [] if cfg.sel_dram else [(dstsel_sb, dstsel_d)]):
                nc.sync.dma_start(sb[:], d[:])

            # ---- DRAM internals ----
            # Shared tensors are single-writer in Tile scheduling, so
            # allocate fresh ones per repeat iteration.
            ht_in = [dpool.tile([SH, RF], F32, name=f"ht{i}_in") for i in (1, 2)]
            htc = [None, None]
            bn_in = [dpool.tile([1, 2 * F], F32, name=f"bn{i}_in") for i in (1, 2)]
            bn_out = [None, None]

            def alloc_shared(rep):
                for i in (0, 1):
                    htc[i] = [dpool.tile([cfg.crows, RF], F32,
                                         addr_space=shared_as,
                                         name=f"ht{i + 1}_r{rep}_c{c}")
                              for c in range(CC)]
                    bn_out[i] = dpool.tile([1, 2 * F], F32, addr_space=shared_as,
                                           name=f"bn{i + 1}_out_r{rep}")

            fc_in = dpool.tile([cfg.k, GP], F32)
            fc_out = dpool.tile([cfg.k, GP], F32, addr_space=shared_as)

            # ================= helper phases =================

            def dense_phase(li, wext_sb):
                """h_cur -> table rows (ht_in) + ar_sb; chunked AllGather."""
                RIN = cfg.rin
                for b in range(NB):
                    tr_ps = ps_pool.tile([128, 128], F32, tag="psA")
                    nc.tensor.transpose(tr_ps[:], h_cur[:, b, :], ident[:])
                    xT = wp_pool.tile([128, 128], F32, tag="xT")
                    nc.scalar.activation(xT[:], tr_ps[:], AF.Copy)
                    dp_ps = ps_pool.tile([128, F + 2 * H], F32, tag="psB")
                    nc.tensor.matmul(dp_ps[:], xT[:], wext_sb[:], start=True,
                                     stop=True)
                    row = wp_pool.tile([128, RF], F32, tag="row")
                    nc.scalar.activation(row[:, :64].bitcast(BF), dp_ps[:, :F],
                                         AF.Copy)
                    nc.vector.tensor_copy(row[:, 64:64 + H], dp_ps[:, F:FH])
                    nc.vector.memset(row[:, 64 + H:], 0.0)
                    nc.vector.tensor_copy(ar_sb[:, b, :], dp_ps[:, FH:FH + H])
                    nc.sync.dma_start(ht_in[li][b * 128:(b + 1) * 128, :], row[:])
                if not cfg.no_ag:
                    for c in range(CC):
                        nc.gpsimd.collective_compute(
                            "AllGather", AX.bypass, replica_groups=rg,
                            ins=[ht_in[li][c * RIN:(c + 1) * RIN, :]],
                            outs=[htc[li][c][:]])

            gq = [0]  # global SWDGE gather counter: keeps sem slot <-> queue
                      # binding stable (slot = idx%8, queue = idx%4)

            def scatter_phase(li, bbc_sb):
                """edge phase: gathers + selector matmuls -> h_cur (+bias)."""
                for b in range(NB):
                    TB = cfg.tb[b]
                    ioff = cfg.ioff[b]
                    idxt = gp_pool.tile([128, TBM * 8], I16, tag="idx")
                    nc.sync.dma_start(idxt[:, :TB * 8],
                                      idx_flat_d[:, ioff * 8:(ioff + TB) * 8])
                    glo = gp_pool.tile([128, TBM, RF], F32, tag="glo")
                    off = 0
                    for c in range(CC):
                        t = cfg.tbc[b][c]
                        if t == 0:
                            continue
                        nreal = cfg.cbc[b][c] if cfg.neg_pad else t * 128
                        nc.gpsimd.dma_gather(
                            out_ap=glo[:, off:off + t, :],
                            in_ap=htc[li][c][:],
                            idxs_ap=idxt[:, off * 8:(off + t) * 8],
                            num_idxs=t * 128, num_idxs_reg=nreal,
                            elem_size=RF,
                            queue_num=gq[0] % 4, single_packet=False)
                        gq[0] += 1
                        off += t
                    # selectors: sel[e, t, c] = (dst(b,t,e) == c)
                    sel = wp_pool.tile([128, TBM, 128], BF, tag="sel")
                    if cfg.sel_dram:
                        nc.sync.dma_start(sel[:, :TB, :],
                                          selp_d[b, :, :TB * 128].rearrange(
                                              "p (t c) -> p t c", c=128))
                    else:
                        nc.vector.tensor_tensor(
                            sel[:, :TB, :],
                            dstsel_sb[:, b, :TB].unsqueeze(2).broadcast_to(
                                [128, TB, 128]),
                            diota[:].unsqueeze(1).broadcast_to([128, TB, 128]),
                            AX.is_equal)
                    drow = gp_pool.tile([128, TBM * 128], I8, tag="drow")
                    nc.sync.dma_start(drow[:, :TB * 128],
                                      dstrow_d[b, :, :TB * 128])
                    selT = wp_pool.tile([128, TBM * 128], BF, tag="selT")
                    eng = (nc.gpsimd if cfg.pool_selt and b % cfg.pool_selt
                           else nc.vector)
                    eng.tensor_scalar(selT[:, :TB * 128],
                                      drow[:, :TB * 128], piota[:], None,
                                      AX.is_equal)
                    # ar expand per tile
                    arx_ps = ps_pool.tile([128, TBM, H], F32, tag="psA")
                    for t in range(TB):
                        nc.tensor.matmul(arx_ps[:, t, :],
                                         selT[:, t * 128:(t + 1) * 128],
                                         ar_sb[:, b, :], start=True, stop=True)
                    # e = al + ar ; lrelu ; exp -> p
                    wpt = wp_pool.tile([128, TBM, FH], BF, tag="wpt")
                    e_sb = wp_pool.tile([128, TBM, H], F32, tag="e")
                    nc.vector.tensor_tensor(e_sb[:, :TB, :],
                                            glo[:, :TB, ALO:ALO + H],
                                            arx_ps[:, :TB, :], AX.add)
                    # leaky relu in one op: max(slope*e, e)
                    eneg = wp_pool.tile([128, TBM, H], F32, tag="eneg")
                    nc.vector.scalar_tensor_tensor(
                        eneg[:, :TB, :], e_sb[:, :TB, :], cfg.neg_slope,
                        e_sb[:, :TB, :], AX.mult, AX.max)
                    nc.scalar.activation(wpt[:, :TB, F:FH], eneg[:, :TB, :],
                                         AF.Exp)
                    # p expanded across channels on Act -> stride-1 2x multiply
                    glo_h = glo[:, :, :64].bitcast(BF)
                    pexp = wp_pool.tile([128, TBM, F], BF, tag="pexp")
                    nc.scalar.activation(
                        pexp[:, :TB, :].rearrange("p t (h c) -> p t h c", c=C_),
                        wpt[:, :TB, F:FH].unsqueeze(3).broadcast_to(
                            [128, TB, H, C_]),
                        AF.Copy)
                    nc.vector.tensor_tensor(wpt[:, :TB, :F], glo_h[:, :TB, :],
                                            pexp[:, :TB, :], AX.mult)
                    # scatter matmuls
                    acc_ps = ps_pool.tile([128, FH], F32, tag="psB")
                    for t in range(TB):
                        nc.tensor.matmul(acc_ps[:], sel[:, t, :], wpt[:, t, :],
                                         start=(t == 0), stop=(t == TB - 1))
                    # divide + bias -> h_cur
                    s_sb = wp_pool.tile([128, H], F32, tag="s")
                    nc.vector.tensor_scalar(s_sb[:], acc_ps[:, F:FH], 1e-30,
                                            None, AX.max)
                    r_sb = wp_pool.tile([128, H], F32, tag="r")
                    nc.vector.reciprocal(r_sb[:], s_sb[:])
                    nc.vector.tensor_tensor(
                        h_cur[:, b, :].rearrange("p (h c) -> p h c", c=C_),
                        acc_ps[:, :F].rearrange("p (h c) -> p h c", c=C_),
                        r_sb[:].unsqueeze(2).broadcast_to([128, H, C_]),
                        AX.mult)
                    nc.vector.tensor_tensor(h_cur[:, b, :], h_cur[:, b, :],
                                            bbc_sb[:], AX.add)

            def bn_elu_phase(li, g_sb, be_sb):
                bn_ps = ps1_pool.tile([1, 2 * F], F32, tag="ps1")
                for b in range(NB):
                    rhs = wp_pool.tile([128, 2 * F], F32, tag="bnrhs")
                    nc.vector.tensor_scalar(rhs[:, :F], h_cur[:, b, :],
                                            mask_sb[:, b].unsqueeze(1), None,
                                            AX.mult)
                    nc.scalar.activation(rhs[:, F:], rhs[:, :F], AF.Square)
                    nc.tensor.matmul(bn_ps[:], onescol[:], rhs[:],
                                     start=(b == 0), stop=(b == NB - 1))
                bn_sb = wp_pool.tile([1, 2 * F], F32, tag="bnrow")
                nc.vector.tensor_copy(bn_sb[:], bn_ps[:])
                nc.sync.dma_start(bn_in[li][:], bn_sb[:])
                nc.gpsimd.collective_compute(
                    "AllReduce", AX.add, replica_groups=rg,
                    ins=[bn_in[li][:]], outs=[bn_out[li][:]])
                st = wp_pool.tile([1, 2 * F], F32, tag="bnst")
                nc.sync.dma_start(st[:], bn_out[li][:])
                # mu = s/n ; var = ss/n - mu^2
                mu = wp_pool.tile([1, F], F32, tag="mu")
                nc.vector.tensor_scalar(mu[:], st[:, :F], 1.0 / cfg.n_real,
                                        None, AX.mult)
                var = wp_pool.tile([1, F], F32, tag="var")
                nc.vector.tensor_scalar(var[:], st[:, F:], 1.0 / cfg.n_real,
                                        None, AX.mult)
                mu2 = wp_pool.tile([1, F], F32, tag="mu2")
                nc.scalar.activation(mu2[:], mu[:], AF.Square)
                nc.vector.tensor_tensor(var[:], var[:], mu2[:], AX.subtract)
                # rstd = 1/sqrt(var+eps)
                nc.vector.tensor_scalar(var[:], var[:], cfg.eps, None, AX.add)
                sd = wp_pool.tile([1, F], F32, tag="sd")
                nc.scalar.activation(sd[:], var[:], AF.Sqrt)
                rstd = wp_pool.tile([1, F], F32, tag="rstd")
                nc.vector.reciprocal(rstd[:], sd[:])
                # scale = g*rstd ; shift = be - mu*scale
                ssrow = wp_pool.tile([1, 2 * F], F32, tag="ssrow")
                nc.vector.tensor_tensor(ssrow[:, :F], g_sb[:], rstd[:], AX.mult)
                musc = wp_pool.tile([1, F], F32, tag="musc")
                nc.vector.tensor_tensor(musc[:], mu[:], ssrow[:, :F], AX.mult)
                nc.vector.tensor_tensor(ssrow[:, F:], be_sb[:], musc[:],
                                        AX.subtract)
                # broadcast via K=1 matmul
                bc_ps = ps1_pool.tile([128, 2 * F], F32, tag="ps1")
                nc.tensor.matmul(bc_ps[:], onesrow[:], ssrow[:], start=True,
                                 stop=True)
                bc_sb = wp_pool.tile([128, 2 * F], F32, tag="bnbcsb")
                nc.vector.tensor_copy(bc_sb[:], bc_ps[:])
                # normalize + elu
                for b in range(NB):
                    nc.vector.tensor_tensor(h_cur[:, b, :], h_cur[:, b, :],
                                            bc_sb[:, :F], AX.mult)
                    nc.vector.tensor_tensor(h_cur[:, b, :], h_cur[:, b, :],
                                            bc_sb[:, F:], AX.add)
                    neg = wp_pool.tile([128, F], F32, tag="neg")
                    nc.vector.tensor_scalar(neg[:], h_cur[:, b, :], 0.0, None,
                                            AX.min)
                    ex = wp_pool.tile([128, F], F32, tag="ex")
                    nc.scalar.activation(ex[:], neg[:], AF.Exp)
                    nc.vector.tensor_scalar(h_cur[:, b, :], h_cur[:, b, :],
                                            0.0, -1.0, AX.max, AX.add)
                    nc.vector.tensor_tensor(h_cur[:, b, :], h_cur[:, b, :],
                                            ex[:], AX.add)

            # ================= program =================
            for _rep in range(cfg.repeat):
              alloc_shared(_rep)
              dense_phase(0, w1e_sb)
              if cfg.stage >= 2:
                scatter_phase(0, b1bc_sb)
              if cfg.stage >= 3:
                bn_elu_phase(0, g1_sb, be1_sb)
              if cfg.stage >= 4:
                dense_phase(1, w2e_sb)
              if cfg.stage >= 5:
                scatter_phase(1, b2bc_sb)
                bn_elu_phase(1, g2_sb, be2_sb)
            if cfg.stage < 6:
                dbg = wp_pool.tile([cfg.k, 128], F32, tag="dbg")
                nc.vector.tensor_copy(dbg[:], h_cur[0:cfg.k, 0, :])
                nc.sync.dma_start(out_d[:, :128], dbg[:])
            _full = cfg.stage >= 6
            # pooling
            try:
                pool_ps = ps1_pool.tile([128, GP], F32, tag="ps1")
                for b in range(NB if _full else 0):
                    gsel = wp_pool.tile([128, GP], F32, tag="gsel")
                    nc.vector.tensor_scalar(gsel[:], giota[:],
                                            bcol_sb[:, b].unsqueeze(1), None,
                                            AX.is_equal)
                    nc.tensor.matmul(pool_ps[:], h_cur[:, b, :], gsel[:],
                                     start=(b == 0), stop=(b == NB - 1))
                if not _full:
                    raise _SkipRest
                pool_sb = wp_pool.tile([128, GP], F32, tag="poolsb")
                nc.vector.tensor_copy(pool_sb[:], pool_ps[:])
                fc_ps = ps1_pool.tile([cfg.k, GP], F32, tag="ps1")
                nc.tensor.matmul(fc_ps[:], fcw_sb[:], pool_sb[:], start=True,
                                 stop=True)
                fc_sb = wp_pool.tile([cfg.k, GP], F32, tag="fcsb")
                nc.vector.tensor_copy(fc_sb[:], fc_ps[:])
                nc.sync.dma_start(fc_in[:], fc_sb[:])
                nc.gpsimd.collective_compute("AllReduce", AX.add,
                                             replica_groups=rg,
                                             ins=[fc_in[:]], outs=[fc_out[:]])
                fin = wp_pool.tile([cfg.k, GP], F32, tag="fin")
                nc.sync.dma_start(fin[:], fc_out[:])
                nc.vector.tensor_tensor(fin[:], fin[:], rcnt_sb[:], AX.mult)
                nc.vector.tensor_tensor(fin[:], fin[:], fcbbc_sb[:], AX.add)
                nc.sync.dma_start(out_d[:], fin[:])
            except _SkipRest:
                pass

    nc.compile()
    return nc


# ---------------------------------------------------------------------------
# harness entry point: full inputs in, full output out
# ---------------------------------------------------------------------------

_NC_CACHE = {}


def kernel(**inputs):
    """Full-input GAT forward on 8 NeuronCores. Returns [500, 6] float32."""
    from concourse.bass_utils import run_bass_kernel_spmd

    cfg = Cfg()
    in_maps = prep_inputs(cfg, inputs)
    key = (cfg.tbc, cfg.ag_chunks)
    if key not in _NC_CACHE:
        _NC_CACHE[key] = build_nc(cfg)
    nc = _NC_CACHE[key]
    res = run_bass_kernel_spmd(nc, in_maps, core_ids=list(range(cfg.ncores)))
    out = res.results[0]["out"]
    return np.ascontiguousarray(out[:, :cfg.g].T).astype(np.float32)

